# revision 17
# baseline (speedup 1.0000x reference)
"""Trainium2 Bass kernel: batched locally-weighted ridge regression.

Per test point t: K[t,n] = exp(-|xte_t - xtr_n|^2 / (2 ls^2));
  A_t = Xtild^T diag(K[t]) Xtild + REG*I ; b_t = Xtild^T (K[t] * Y)
  ypred_t = xtild_t . A_t^{-1} b_t
Sharding: data-parallel over the 4096 test points -> 8 cores x 512.

On-device math uses a scaled kernel K'[t,n] = exp((S[n,t] - sn[n]/2) * c2)
(c2 = 1/ls^2), i.e. the exp(-st*c2/2) per-test factor is dropped; this
rescales A_t and b_t identically, so beta is preserved by using a
per-test ridge REG_t = REG * exp(st*c2/2).

Pipeline per core:
  PE : 16 transposes, gram S = Xtr @ Xte^T, big matmul K'^T-chunks @ [Z | Xtild*Y]
  ACT: fused exp(S*c2 - sn*c2/2), PSUM evacuations, copies
  DVE: Z build (outer products via stride-0 APs), batched Gaussian
       elimination + back-substitution (batch on partitions, 4 blocks of
       128 systems in the free dim), predictions.
"""

import numpy as np

import concourse.bacc as bacc
import concourse.bass as bass
import concourse.mybir as mybir
from concourse.bass import ds, ts
from concourse.bass_utils import run_bass_kernel_spmd
from concourse.tile import TileContext

F32 = mybir.dt.float32
P = 128
N_TRAIN = 2048
D = 31
DP = 32          # 1 + D
W = 33           # DP + rhs column
N_TEST = 4096
NCORES = 8
TS = N_TEST // NCORES   # 512 test points per core
NT = TS // P            # 4 t-tiles
NK = N_TRAIN // P       # 16 train chunks
REG = 1e-6
LNREG = float(np.log(REG))
F32R = mybir.dt.float32r
MM_FP32R = False     # f32r measured 1.1e-2 rel err on HW (vs 3.6e-5 fp32)


def _build_nc(c2: float):
    """Build the single-core Bass program (SPMD across 8 cores)."""
    nc = bacc.Bacc(trn_type="TRN2")

    xtr_d = nc.dram_tensor("xtrain", [N_TRAIN, D], F32, kind="ExternalInput")
    ytr_d = nc.dram_tensor("ytrain", [N_TRAIN, 1], F32, kind="ExternalInput")
    xte_d = nc.dram_tensor("xtest", [TS, D], F32, kind="ExternalInput")
    # transposed train || test features, one DMA -> one wait for the PE
    xT_d = nc.dram_tensor("xT", [D, N_TRAIN + TS], F32, kind="ExternalInput")
    out_d = nc.dram_tensor("ypred", [TS, 1], F32, kind="ExternalOutput")

    c2h = 0.5 * c2

    with TileContext(nc) as tc:
        with (
            tc.tile_pool(name="const", bufs=1) as const,
            tc.tile_pool(name="sb", bufs=1) as sb,
            tc.tile_pool(name="pgram", bufs=2, space="PSUM") as pgram,
            tc.tile_pool(name="pxwx", bufs=4, space="PSUM") as pxwx,
        ):
            # ---- load inputs ----
            xtr = sb.tile([P, NK, D], F32)       # natural layout chunks
            nc.sync.dma_start(
                xtr, xtr_d.rearrange("(c p) d -> p c d", p=P)
            )
            ytr = sb.tile([P, NK], F32)
            nc.sync.dma_start(
                ytr, ytr_d.rearrange("(c p) one -> p (c one)", p=P)
            )
            xte = sb.tile([P, NT, D], F32)
            nc.sync.dma_start(
                xte, xte_d.rearrange("(t p) d -> p t d", p=P)
            )

            # ---- transposed inputs: [XtrT | XteT] = [31, 2048+512] ----
            xT = sb.tile([DP, N_TRAIN + TS], F32)
            nc.sync.dma_start(xT[:D], xT_d[:, :])
            xtrT = xT[:, 0:N_TRAIN].rearrange("d (c p) -> d c p", p=P)
            xteT = xT[:, N_TRAIN:]

            # ---- Xtild chunks [128, NK, 32] (ones column + Xtrain) ----
            xt = sb.tile([P, NK, DP], F32)
            nc.vector.memset(xt[:, :, 0:1], 1.0)
            nc.scalar.copy(xt[:, :, 1:DP], xtr)

            # ---- Z = [xtild_d * xtild_e (1024) | xtild * y (32)] ----
            MMDT = F32R if MM_FP32R else F32
            H = 16
            NZ = DP * H + H * H + DP             # 512 + 256 + 32 = 800
            zz = sb.tile([P, NK, NZ], MMDT)
            for c in range(NK):
                nc.vector.tensor_mul(
                    zz[:, c, 0:DP * H].rearrange("p (d e) -> p d e", d=DP),
                    xt[:, c, :, None].broadcast_to([P, DP, H]),
                    xt[:, c, None, H:DP].broadcast_to([P, DP, H]),
                )
                nc.vector.tensor_mul(
                    zz[:, c, DP * H:DP * H + H * H].rearrange(
                        "p (d e) -> p d e", d=H),
                    xt[:, c, 0:H, None].broadcast_to([P, H, H]),
                    xt[:, c, None, 0:H].broadcast_to([P, H, H]),
                )
                nc.vector.tensor_scalar_mul(
                    zz[:, c, DP * H + H * H:], xt[:, c, :], ytr[:, ds(c, 1)]
                )

            # ---- squared norms and per-partition exp biases ----
            sqtr = sb.tile([P, NK, D], F32)
            sn = sb.tile([P, NK], F32)
            nc.vector.tensor_mul(sqtr, xtr, xtr)
            nc.vector.tensor_reduce(
                sn, sqtr, mybir.AxisListType.X, mybir.AluOpType.add,
            )
            sqte = sb.tile([P, NT, D], F32)
            st = sb.tile([P, NT], F32)
            nc.vector.tensor_mul(sqte, xte, xte)
            nc.vector.tensor_reduce(
                st, sqte, mybir.AxisListType.X, mybir.AluOpType.add,
            )
            bias_n = sb.tile([P, NK], F32)       # -sn * c2/2
            nc.vector.tensor_scalar_mul(bias_n, sn, -c2h)
            # per-test ridge REG_t = exp(st*c2/2 + ln(REG)), [128, NT]
            lnreg_t = const.tile([P, 1], F32)
            nc.vector.memset(lnreg_t, LNREG)
            regt = sb.tile([P, NT], F32)
            nc.scalar.activation(
                regt, st, mybir.ActivationFunctionType.Exp,
                bias=lnreg_t[:, :], scale=c2h,
            )

            # ---- gram + K' = exp(S*c2 - sn*c2/2), layout [n_chunk, t] ----
            kp = sb.tile([P, NK, TS], MMDT)
            for c in range(NK):
                sg = pgram.tile([P, TS], F32, tag="sg")
                nc.tensor.matmul(sg, xtrT[:D, c, :], xteT[:D, :],
                                 start=True, stop=True)
                nc.scalar.activation(
                    kp[:, c, :], sg, mybir.ActivationFunctionType.Exp,
                    bias=bias_n[:, ds(c, 1)], scale=c2,
                )

            # ---- XWX | XWy: [512, 1056] per core via K'-chunks @ ZZ ----
            # ga holds [A | b] per system: [128 part(t), NT blocks, 32 rows, 33 cols]
            ga = sb.tile([P, NT, DP, W], F32)
            CHUNKS = [(0, 512), (512, 800)]
            for t in range(NT):
                for (c0, c1) in CHUNKS:
                    w = c1 - c0
                    px = pxwx.tile([P, 512], F32, tag="px")
                    for c in range(NK):
                        nc.tensor.matmul(
                            px[:, :w],
                            kp[:, c, ts(t, P)],
                            zz[:, c, c0:c1],
                            start=(c == 0), stop=(c == NK - 1),
                        )
                    if c0 == 0:
                        # cols e=16..31, all rows d
                        nc.scalar.copy(
                            ga[:, t, :, H:DP],
                            px[:, :w].rearrange("p (r c) -> p r c", r=DP),
                        )
                    else:
                        # top-left quadrant + rhs column
                        nc.scalar.copy(
                            ga[:, t, 0:H, 0:H],
                            px[:, 0:H * H].rearrange("p (r c) -> p r c", r=H),
                        )
                        nc.scalar.copy(ga[:, t, :, DP], px[:, H * H:H * H + DP])

            # mirror lower-left quadrant from upper-right (A symmetric)
            ga_sw = ga[:].rearrange("p b r c -> p b c r")
            for b0 in (0, 2):
                nc.scalar.copy(
                    ga[:, b0:b0 + 2, H:DP, 0:H],
                    ga_sw[:, b0:b0 + 2, H:DP, 0:H],
                )

            # ---- add per-test ridge on the diagonal (per 2-block half) ----
            ga_flat = ga[:].rearrange("p b r c -> p b (r c)")
            ga_diag = ga_flat[:, :, ::W + 1]     # [128, NT, 32]
            for b0 in (0, 2):
                nc.vector.tensor_add(
                    ga_diag[:, b0:b0 + 2], ga_diag[:, b0:b0 + 2],
                    regt[:, b0:b0 + 2, None].broadcast_to([P, 2, DP]),
                )

            # ---- batched Gaussian elimination (no pivoting; A is SPD) ----
            # two independent 2-block halves so the scheduler overlaps the
            # first half's elimination with the second half's XWX matmuls
            invp = sb.tile([P, NT, DP], F32)
            fbuf0 = sb.tile([P, 2, D], F32)
            tbuf0 = sb.tile([P, 2, D, DP], F32)
            fbuf1 = sb.tile([P, 2, D], F32)
            tbuf1 = sb.tile([P, 2, D, DP], F32)
            for b0, b1, fbuf, tbuf in ((0, 2, fbuf0, tbuf0),
                                       (2, 4, fbuf1, tbuf1)):
                nb = b1 - b0
                for k in range(DP):
                    nc.vector.reciprocal(
                        invp[:, b0:b1, k], ga[:, b0:b1, k, k])
                    if k == DP - 1:
                        break
                    m = D - k          # rows k+1..31
                    w = DP - k         # cols k+1..32 (incl. rhs)
                    nc.vector.tensor_mul(
                        fbuf[:, :, :m],
                        ga[:, b0:b1, k + 1:DP, k],
                        invp[:, b0:b1, k:k + 1].broadcast_to([P, nb, m]),
                    )
                    nc.vector.tensor_mul(
                        tbuf[:, :, :m, :w],
                        fbuf[:, :, :m, None].broadcast_to([P, nb, m, w]),
                        ga[:, b0:b1, k:k + 1, k + 1:W].broadcast_to(
                            [P, nb, m, w]),
                    )
                    nc.vector.tensor_sub(
                        ga[:, b0:b1, k + 1:DP, k + 1:W],
                        ga[:, b0:b1, k + 1:DP, k + 1:W],
                        tbuf[:, :, :m, :w],
                    )

            # ---- back-substitution ----
            xsol = sb.tile([P, NT, DP], F32)
            dotb = sb.tile([P, NT], F32)
            tmp2 = sb.tile([P, NT], F32)
            bsc = sb.tile([P, NT, D], F32)
            nc.vector.tensor_mul(
                xsol[:, :, DP - 1], ga[:, :, DP - 1, DP], invp[:, :, DP - 1]
            )
            for k in range(DP - 2, -1, -1):
                m = D - k          # solved entries k+1..31
                nc.vector.tensor_mul(
                    bsc[:, :, :m], ga[:, :, k, k + 1:DP], xsol[:, :, k + 1:DP]
                )
                nc.vector.tensor_reduce(
                    dotb, bsc[:, :, :m],
                    mybir.AxisListType.X, mybir.AluOpType.add,
                )
                nc.vector.tensor_sub(tmp2, ga[:, :, k, DP], dotb)
                nc.vector.tensor_mul(xsol[:, :, k], tmp2, invp[:, :, k])

            # ---- predictions: ypred = xtild_test . beta ----
            xtt = sb.tile([P, NT, DP], F32)
            nc.vector.memset(xtt[:, :, 0:1], 1.0)
            nc.scalar.copy(xtt[:, :, 1:DP], xte)
            yp = sb.tile([P, NT], F32)
            prod = sb.tile([P, NT, DP], F32)
            nc.vector.tensor_mul(prod, xtt, xsol)
            nc.vector.tensor_reduce(
                yp, prod, mybir.AxisListType.X, mybir.AluOpType.add,
            )
            nc.sync.dma_start(
                out_d.rearrange("(t p) one -> p (t one)", p=P), yp
            )

    nc.finalize()
    return nc


_cache: dict[float, object] = {}


def _get_nc(c2: float):
    if c2 not in _cache:
        _cache[c2] = _build_nc(c2)
    return _cache[c2]


def kernel(Ytrain, Xtrain, Xtest, log_lengthscale, _trace=False):
    Ytrain = np.ascontiguousarray(np.asarray(Ytrain, dtype=np.float32))
    Xtrain = np.ascontiguousarray(np.asarray(Xtrain, dtype=np.float32))
    Xtest = np.ascontiguousarray(np.asarray(Xtest, dtype=np.float32))
    lls = float(np.asarray(log_lengthscale, dtype=np.float32))
    c2 = float(np.exp(np.float32(-2.0 * lls)))

    nc = _get_nc(c2)
    in_maps = []
    for core in range(NCORES):
        shard = np.ascontiguousarray(Xtest[core * TS:(core + 1) * TS])
        in_maps.append({
            "xtrain": Xtrain,
            "ytrain": Ytrain,
            "xtest": shard,
            "xT": np.ascontiguousarray(
                np.concatenate([Xtrain.T, shard.T], axis=1)),
        })
    res = run_bass_kernel_spmd(nc, in_maps, list(range(NCORES)),
                               trace=bool(_trace))
    outs = [np.asarray(res.results[c]["ypred"], dtype=np.float32)
            for c in range(NCORES)]
    full = np.concatenate(outs, axis=0)
    if _trace:
        return full, res
    return full
